# revision 3
# baseline (speedup 1.0000x reference)
"""Trainium2 Bass kernel for nn_HardwareOptimizedSpikeProcessor.

Reference semantics (per timestep t):
    acc += (s_t @ (W*mask).T) * 2**scale_exp     # [B, Cout]
    spk  = acc >= 2**threshold_exp
    acc  = acc * (1 - spk)
    out[:, :, t] = spk

Strategy (v2):
  - Shard batch/2 x cout/4: each of the 8 cores handles 32 samples x 512
    output channels.  PE work per core is unchanged (8.6 GFLOP bf16), but the
    matmul free dim is b*tb = 32*8 = 256 at a t-block of only 8 steps, which
    keeps LDWEIGHTS fully amortized while letting the sequential scan pipeline
    against the PE at fine (8-step) granularity -- the scan tail after the
    last matmul shrinks from ~60us (baseline) to ~6us.
  - The matmul contribution c[t] = s_t @ Wm.T is exact in bf16 (spikes are
    0/1; masked weights are ints in [-127,127]); PSUM accumulates fp32.
  - Scan step is 2 DVE instructions instead of 3:
        u_t  = acc + c_t                         (tensor_tensor add)
        acc  = (u_t < thr) * u_t                 (scalar_tensor_tensor)
    and spikes (u_t >= thr) are extracted in bulk per 8-step block on the
    otherwise-idle Pool engine, off the serial chain.
  - PE p-state warm-up: ~3us of junk matmuls issued while the first DMAs
    land, so real matmuls run at full clock from the start.
"""

import sys

for _p in ("/opt/trn_rl_repo",):
    if _p not in sys.path:
        sys.path.insert(0, _p)

import numpy as np
import ml_dtypes

import concourse.bass as bass
import concourse.mybir as mybir
import concourse.tile as tile
from concourse.bass_utils import run_bass_kernel_spmd

B, CIN, COUT, T = 64, 2048, 2048, 128
NCORES = 8
NB = 2                      # batch shards
NQ = 4                      # cout shards
BLOC = B // NB              # 32 samples per core
QLOC = COUT // NQ           # 512 output channels per core
MC = QLOC // 128            # 4 output-channel chunks per core
KC = CIN // 128             # 16 contraction chunks
TB = 8                      # timesteps per scan block
NBLK = T // TB              # 16 blocks
NFREE = BLOC * TB           # matmul free dim = 256

# spike DMA chunks along T (first small so PE starts early)
TCHUNKS = [8, 8, 16, 32, 32, 32]
assert sum(TCHUNKS) == T

_MAX_WAITS = 1


def _split_excess_waits(nc):
    """This container's walrus build accepts at most one sync-wait per
    instruction; spill extra waits onto same-engine NOPs placed before the
    offending instruction."""
    for f in nc.m.functions:
        for bb in f.blocks:
            new_list = []
            for ins in bb.instructions:
                si = ins.sync_info
                waits = list(si.on_wait) if si is not None and si.on_wait else []
                if len(waits) > _MAX_WAITS:
                    extra, keep = waits[:-_MAX_WAITS], waits[-_MAX_WAITS:]
                    for i in range(0, len(extra), _MAX_WAITS):
                        nop = mybir.InstNoOp(
                            name=f"{ins.name}-waitsplit-{i}", ins=[], outs=[]
                        )
                        nop.engine = ins.engine
                        nop.sync_info = mybir.SyncInfo(
                            on_wait=extra[i : i + _MAX_WAITS], on_update=[]
                        )
                        new_list.append(nop)
                    ins.sync_info = mybir.SyncInfo(
                        on_wait=keep,
                        on_update=list(si.on_update) if si.on_update else [],
                    )
                new_list.append(ins)
            bb.instructions[:] = new_list


def _build(thr: float):
    f32 = mybir.dt.float32
    bf16 = mybir.dt.bfloat16
    u8 = mybir.dt.uint8
    nc = bass.Bass()

    # W^T (2**scale_exp folded in): per m-chunk [cin_lo, k, cout_lo]
    wt_ds = [
        nc.dram_tensor(f"wt{m}", [128, KC, 128], bf16, kind="ExternalInput")
        for m in range(MC)
    ]
    # spike chunks, each contiguous [cin_lo, k, b, tc]
    spk_ds = [
        nc.dram_tensor(f"spk{j}", [128, KC, BLOC, tc], bf16, kind="ExternalInput")
        for j, tc in enumerate(TCHUNKS)
    ]
    # per-block spike outputs [cout_lo, t, m, b]
    out_ds = [
        nc.dram_tensor(f"out{j}", [128, TB, MC, BLOC], u8, kind="ExternalOutput")
        for j in range(NBLK)
    ]

    # block -> (chunk index, t offset within chunk)
    blk_map = []
    cj, coff = 0, 0
    for j in range(NBLK):
        if coff >= TCHUNKS[cj]:
            cj += 1
            coff = 0
        blk_map.append((cj, coff))
        coff += TB

    with tile.TileContext(nc) as tc:
        with (
            tc.tile_pool(name="const", bufs=1) as const,
            tc.tile_pool(name="cpool", bufs=3) as cpool,
            tc.tile_pool(name="upool", bufs=3) as upool,
            tc.tile_pool(name="opool", bufs=3) as opool,
            tc.tile_pool(name="psum", bufs=3, space="PSUM") as psum,
            tc.tile_pool(name="pwarm", bufs=1, space="PSUM") as pwarm,
        ):
            wt_sb = const.tile([128, MC, KC, 128], bf16)
            spk_sbs = [
                const.tile([128, KC, BLOC, tc], bf16, name=f"spk_sb{j}")
                for j, tc in enumerate(TCHUNKS)
            ]
            acc = const.tile([128, MC, BLOC], f32)
            junk = const.tile([128, 256], bf16)

            nc.vector.memset(acc[:], 0.0)
            nc.vector.memset(junk[:], 0.0)

            # DMA order: first spike chunk + weights first so PE starts ASAP
            nc.sync.dma_start(spk_sbs[0][:], spk_ds[0][:])
            for m in range(MC):
                nc.sync.dma_start(wt_sb[:, m], wt_ds[m][:])
            for j in range(1, len(TCHUNKS)):
                nc.sync.dma_start(spk_sbs[j][:], spk_ds[j][:])

            # PE p-state warm-up on junk data while the DMAs land (~3us)
            wps = pwarm.tile([128, 256], f32)
            for _ in range(28):
                nc.tensor.matmul(wps[:], lhsT=junk[:, :128], rhs=junk[:])

            for j in range(NBLK):
                cj, toff = blk_map[j]
                ps = psum.tile([128, MC, NFREE], f32, tag="ps", name="ps")
                for m in range(MC):
                    for k in range(KC):
                        nc.tensor.matmul(
                            ps[:, m, :],
                            lhsT=wt_sb[:, m, k, :],
                            rhs=spk_sbs[cj][:, k, :, toff : toff + TB],
                            start=(k == 0),
                            stop=(k == KC - 1),
                        )
                # PSUM [p, m, (b t)] -> SBUF c [p, t, m, b] so each scan step
                # reads a contiguous [128, (m b)] slice
                c = cpool.tile([128, TB, MC, BLOC], f32, tag="cblk")
                nc.scalar.copy(
                    c[:].rearrange("p t m b -> p m b t"),
                    ps[:].rearrange("p m (b t) -> p m b t", b=BLOC),
                )
                u = upool.tile([128, TB, MC, BLOC], f32, tag="ublk")
                for t in range(TB):
                    nc.vector.tensor_tensor(
                        u[:, t], acc[:], c[:, t], mybir.AluOpType.add
                    )
                    nc.vector.scalar_tensor_tensor(
                        acc[:], u[:, t], thr, u[:, t],
                        mybir.AluOpType.is_lt, mybir.AluOpType.mult,
                    )
                # spikes = (u >= thr), off the serial chain (Pool; last block
                # on DVE to avoid a cross-engine handoff at the very end)
                ob = opool.tile([128, TB, MC, BLOC], u8, tag="oblk")
                eng = nc.vector if j == NBLK - 1 else nc.gpsimd
                eng.tensor_scalar(
                    ob[:], u[:], thr, None, mybir.AluOpType.is_ge
                )
                nc.sync.dma_start(out_ds[j][:], ob[:])

    _split_excess_waits(nc)
    return nc


def _prep_inputs(spikes, weights, mask, scale_exp):
    wm = weights * mask  # integers <= 127, exact
    scale = np.exp2(scale_exp.astype(np.float64)).astype(np.float32)
    wm = (wm * scale[:, None]).astype(np.float32)  # fold power-of-2 scale in
    in_maps = []
    for core in range(NCORES):
        bh, cq = divmod(core, NQ)
        # weights for this cout shard: [qloc, cin] -> W^T -> [m, cin_lo, k, cout_lo]
        wq = wm[cq * QLOC : (cq + 1) * QLOC]  # [512, 2048]
        wt = (
            wq.T.reshape(KC, 128, MC, 128)
            .transpose(2, 1, 0, 3)
            .astype(ml_dtypes.bfloat16)
        )  # [m, cin_lo, k, cout_lo]
        m = {f"wt{mm}": np.ascontiguousarray(wt[mm]) for mm in range(MC)}
        # spikes for this batch shard: [b, cin, t] -> [cin_lo, k, b, t]
        s = spikes[bh * BLOC : (bh + 1) * BLOC]
        a = s.transpose(1, 0, 2).reshape(KC, 128, BLOC, T).transpose(1, 0, 2, 3)
        a = a.astype(ml_dtypes.bfloat16)
        t0 = 0
        for jj, tc in enumerate(TCHUNKS):
            m[f"spk{jj}"] = np.ascontiguousarray(a[:, :, :, t0 : t0 + tc])
            t0 += tc
        in_maps.append(m)
    return in_maps


_CACHE = {}


def _get_program(thr: float):
    if thr not in _CACHE:
        _CACHE[thr] = _build(thr)
    return _CACHE[thr]


def kernel(spikes, weights, mask, scale_exp, threshold_exp, **run_kwargs):
    thr = float(2.0 ** int(np.asarray(threshold_exp)))
    nc = _get_program(thr)
    in_maps = _prep_inputs(
        np.asarray(spikes, dtype=np.float32),
        np.asarray(weights, dtype=np.float32),
        np.asarray(mask, dtype=np.float32),
        np.asarray(scale_exp),
    )
    res = run_bass_kernel_spmd(
        nc, in_maps, core_ids=list(range(NCORES)), **run_kwargs
    )
    full = np.zeros((B, COUT, T), dtype=np.float32)
    for core in range(NCORES):
        bh, cq = divmod(core, NQ)
        blks = [
            np.asarray(res.results[core][f"out{j}"]) for j in range(NBLK)
        ]  # each [cout_lo, t, m, b]
        a = np.concatenate(blks, axis=1)  # [cout_lo, T, m, b]
        # -> [b, m, cout_lo, T] -> [b_loc, qloc, T]
        a = a.transpose(3, 2, 0, 1).reshape(BLOC, QLOC, T)
        full[bh * BLOC : (bh + 1) * BLOC, cq * QLOC : (cq + 1) * QLOC] = a
    if run_kwargs:
        return full, res
    return full


# revision 4
# speedup vs baseline: 2.3148x; 2.3148x over previous
"""Trainium2 Bass kernel for nn_HardwareOptimizedSpikeProcessor.

Reference semantics (per timestep t):
    acc += (s_t @ (W*mask).T) * 2**scale_exp     # [B, Cout]
    spk  = acc >= 2**threshold_exp
    acc  = acc * (1 - spk)
    out[:, :, t] = spk

Strategy (v2):
  - Shard batch/2 x cout/4: each of the 8 cores handles 32 samples x 512
    output channels.  PE work per core is unchanged (8.6 GFLOP bf16), but the
    matmul free dim is b*tb = 32*8 = 256 at a t-block of only 8 steps, which
    keeps LDWEIGHTS fully amortized while letting the sequential scan pipeline
    against the PE at fine (8-step) granularity -- the scan tail after the
    last matmul shrinks from ~60us (baseline) to ~6us.
  - The matmul contribution c[t] = s_t @ Wm.T is exact in bf16 (spikes are
    0/1; masked weights are ints in [-127,127]); PSUM accumulates fp32.
  - Scan step is 2 DVE instructions instead of 3:
        u_t  = acc + c_t                         (tensor_tensor add)
        acc  = (u_t < thr) * u_t                 (scalar_tensor_tensor)
    and spikes (u_t >= thr) are extracted in bulk per 8-step block on the
    otherwise-idle Pool engine, off the serial chain.
  - PE p-state warm-up: ~3us of junk matmuls issued while the first DMAs
    land, so real matmuls run at full clock from the start.
"""

import sys

for _p in ("/opt/trn_rl_repo",):
    if _p not in sys.path:
        sys.path.insert(0, _p)

import numpy as np
import ml_dtypes

import concourse.bass as bass
import concourse.mybir as mybir
import concourse.tile as tile
from concourse.bass_utils import run_bass_kernel_spmd

B, CIN, COUT, T = 64, 2048, 2048, 128
NCORES = 8
NB = 2                      # batch shards
NQ = 4                      # cout shards
BLOC = B // NB              # 32 samples per core
QLOC = COUT // NQ           # 512 output channels per core
MC = QLOC // 128            # 4 output-channel chunks per core
KC = CIN // 128             # 16 contraction chunks
TB = 8                      # timesteps per scan block
NBLK = T // TB              # 16 blocks
NFREE = BLOC * TB           # matmul free dim = 256

# spike DMA chunks along T (first small so PE starts early)
TCHUNKS = [8, 8, 16, 32, 32, 32]
assert sum(TCHUNKS) == T

_MAX_WAITS = 1


def _split_excess_waits(nc):
    """This container's walrus build accepts at most one sync-wait per
    instruction; spill extra waits onto same-engine NOPs placed before the
    offending instruction."""
    for f in nc.m.functions:
        for bb in f.blocks:
            new_list = []
            for ins in bb.instructions:
                si = ins.sync_info
                waits = list(si.on_wait) if si is not None and si.on_wait else []
                if len(waits) > _MAX_WAITS:
                    extra, keep = waits[:-_MAX_WAITS], waits[-_MAX_WAITS:]
                    for i in range(0, len(extra), _MAX_WAITS):
                        nop = mybir.InstNoOp(
                            name=f"{ins.name}-waitsplit-{i}", ins=[], outs=[]
                        )
                        nop.engine = ins.engine
                        nop.sync_info = mybir.SyncInfo(
                            on_wait=extra[i : i + _MAX_WAITS], on_update=[]
                        )
                        new_list.append(nop)
                    ins.sync_info = mybir.SyncInfo(
                        on_wait=keep,
                        on_update=list(si.on_update) if si.on_update else [],
                    )
                new_list.append(ins)
            bb.instructions[:] = new_list


def _build(thr: float):
    f32 = mybir.dt.float32
    bf16 = mybir.dt.bfloat16
    u8 = mybir.dt.uint8
    nc = bass.Bass()

    # W^T (2**scale_exp folded in): per m-chunk [cin_lo, k, cout_lo]
    wt_ds = [
        nc.dram_tensor(f"wt{m}", [128, KC, 128], bf16, kind="ExternalInput")
        for m in range(MC)
    ]
    # spike chunks, each contiguous [cin_lo, k, b, tc]
    spk_ds = [
        nc.dram_tensor(f"spk{j}", [128, KC, BLOC, tc], bf16, kind="ExternalInput")
        for j, tc in enumerate(TCHUNKS)
    ]
    # per-block spike outputs [cout_lo, t, m, b]
    out_ds = [
        nc.dram_tensor(f"out{j}", [128, TB, MC, BLOC], u8, kind="ExternalOutput")
        for j in range(NBLK)
    ]

    # block -> (chunk index, t offset within chunk)
    blk_map = []
    cj, coff = 0, 0
    for j in range(NBLK):
        if coff >= TCHUNKS[cj]:
            cj += 1
            coff = 0
        blk_map.append((cj, coff))
        coff += TB

    with tile.TileContext(nc) as tc:
        with (
            tc.tile_pool(name="const", bufs=1) as const,
            tc.tile_pool(name="cpool", bufs=3) as cpool,
            tc.tile_pool(name="upool", bufs=3) as upool,
            tc.tile_pool(name="opool", bufs=3) as opool,
            tc.tile_pool(name="psum", bufs=3, space="PSUM") as psum,
            tc.tile_pool(name="pwarm", bufs=1, space="PSUM") as pwarm,
        ):
            wt_sb = const.tile([128, MC, KC, 128], bf16)
            spk_sbs = [
                const.tile([128, KC, BLOC, tc], bf16, name=f"spk_sb{j}")
                for j, tc in enumerate(TCHUNKS)
            ]
            acc = const.tile([128, MC, BLOC], f32)
            junk = const.tile([128, 256], bf16)

            nc.vector.memset(acc[:], 0.0)
            nc.vector.memset(junk[:], 0.0)

            # DMA order: first spike chunk + weights first so PE starts ASAP
            nc.sync.dma_start(spk_sbs[0][:], spk_ds[0][:])
            for m in range(MC):
                nc.sync.dma_start(wt_sb[:, m], wt_ds[m][:])
            for j in range(1, len(TCHUNKS)):
                nc.sync.dma_start(spk_sbs[j][:], spk_ds[j][:])

            # PE p-state warm-up on junk data while the DMAs land (~3us)
            wps = pwarm.tile([128, 256], f32)
            for _ in range(28):
                nc.tensor.matmul(wps[:], lhsT=junk[:, :128], rhs=junk[:])

            for j in range(NBLK):
                cj, toff = blk_map[j]
                ps = psum.tile([128, MC, NFREE], f32, tag="ps", name="ps")
                for m in range(MC):
                    for k in range(KC):
                        nc.tensor.matmul(
                            ps[:, m, :],
                            lhsT=wt_sb[:, m, k, :],
                            rhs=spk_sbs[cj][:, k, :, toff : toff + TB],
                            start=(k == 0),
                            stop=(k == KC - 1),
                        )
                # PSUM [p, m, (b t)] -> SBUF c [p, t, m, b] so each scan step
                # reads a contiguous [128, (m b)] slice
                c = cpool.tile([128, TB, MC, BLOC], f32, tag="cblk")
                nc.scalar.copy(
                    c[:].rearrange("p t m b -> p m b t"),
                    ps[:].rearrange("p m (b t) -> p m b t", b=BLOC),
                )
                u = upool.tile([128, TB, MC, BLOC], f32, tag="ublk")
                for t in range(TB):
                    nc.vector.tensor_tensor(
                        u[:, t], acc[:], c[:, t], mybir.AluOpType.add
                    )
                    nc.vector.scalar_tensor_tensor(
                        acc[:], u[:, t], thr, u[:, t],
                        mybir.AluOpType.is_lt, mybir.AluOpType.mult,
                    )
                # spikes = (u >= thr), batched per block off the serial chain
                # (DVE: the Pool engine runs tensor_scalar ~10x slower than
                # its nominal rate, measured 15.6us per block)
                ob = opool.tile([128, TB, MC, BLOC], u8, tag="oblk")
                nc.vector.tensor_scalar(
                    ob[:], u[:], thr, None, mybir.AluOpType.is_ge
                )
                nc.sync.dma_start(out_ds[j][:], ob[:])

    _split_excess_waits(nc)
    return nc


def _prep_inputs(spikes, weights, mask, scale_exp):
    wm = weights * mask  # integers <= 127, exact
    scale = np.exp2(scale_exp.astype(np.float64)).astype(np.float32)
    wm = (wm * scale[:, None]).astype(np.float32)  # fold power-of-2 scale in
    in_maps = []
    for core in range(NCORES):
        bh, cq = divmod(core, NQ)
        # weights for this cout shard: [qloc, cin] -> W^T -> [m, cin_lo, k, cout_lo]
        wq = wm[cq * QLOC : (cq + 1) * QLOC]  # [512, 2048]
        wt = (
            wq.T.reshape(KC, 128, MC, 128)
            .transpose(2, 1, 0, 3)
            .astype(ml_dtypes.bfloat16)
        )  # [m, cin_lo, k, cout_lo]
        m = {f"wt{mm}": np.ascontiguousarray(wt[mm]) for mm in range(MC)}
        # spikes for this batch shard: [b, cin, t] -> [cin_lo, k, b, t]
        s = spikes[bh * BLOC : (bh + 1) * BLOC]
        a = s.transpose(1, 0, 2).reshape(KC, 128, BLOC, T).transpose(1, 0, 2, 3)
        a = a.astype(ml_dtypes.bfloat16)
        t0 = 0
        for jj, tc in enumerate(TCHUNKS):
            m[f"spk{jj}"] = np.ascontiguousarray(a[:, :, :, t0 : t0 + tc])
            t0 += tc
        in_maps.append(m)
    return in_maps


_CACHE = {}


def _get_program(thr: float):
    if thr not in _CACHE:
        _CACHE[thr] = _build(thr)
    return _CACHE[thr]


def kernel(spikes, weights, mask, scale_exp, threshold_exp, **run_kwargs):
    thr = float(2.0 ** int(np.asarray(threshold_exp)))
    nc = _get_program(thr)
    in_maps = _prep_inputs(
        np.asarray(spikes, dtype=np.float32),
        np.asarray(weights, dtype=np.float32),
        np.asarray(mask, dtype=np.float32),
        np.asarray(scale_exp),
    )
    res = run_bass_kernel_spmd(
        nc, in_maps, core_ids=list(range(NCORES)), **run_kwargs
    )
    full = np.zeros((B, COUT, T), dtype=np.float32)
    for core in range(NCORES):
        bh, cq = divmod(core, NQ)
        blks = [
            np.asarray(res.results[core][f"out{j}"]) for j in range(NBLK)
        ]  # each [cout_lo, t, m, b]
        a = np.concatenate(blks, axis=1)  # [cout_lo, T, m, b]
        # -> [b, m, cout_lo, T] -> [b_loc, qloc, T]
        a = a.transpose(3, 2, 0, 1).reshape(BLOC, QLOC, T)
        full[bh * BLOC : (bh + 1) * BLOC, cq * QLOC : (cq + 1) * QLOC] = a
    if run_kwargs:
        return full, res
    return full


# revision 7
# speedup vs baseline: 2.3441x; 1.0127x over previous
"""Trainium2 Bass kernel for nn_HardwareOptimizedSpikeProcessor.

Reference semantics (per timestep t):
    acc += (s_t @ (W*mask).T) * 2**scale_exp     # [B, Cout]
    spk  = acc >= 2**threshold_exp
    acc  = acc * (1 - spk)
    out[:, :, t] = spk

Strategy (v2):
  - Shard batch/2 x cout/4: each of the 8 cores handles 32 samples x 512
    output channels.  PE work per core is unchanged (8.6 GFLOP bf16), but the
    matmul free dim is b*tb = 32*8 = 256 at a t-block of only 8 steps, which
    keeps LDWEIGHTS fully amortized while letting the sequential scan pipeline
    against the PE at fine (8-step) granularity -- the scan tail after the
    last matmul shrinks from ~60us (baseline) to ~6us.
  - The matmul contribution c[t] = s_t @ Wm.T is exact in bf16 (spikes are
    0/1; masked weights are ints in [-127,127]); PSUM accumulates fp32.
  - Scan step is 2 DVE instructions instead of 3:
        u_t  = acc + c_t                         (tensor_tensor add)
        acc  = (u_t < thr) * u_t                 (scalar_tensor_tensor)
    and spikes (u_t >= thr) are extracted in bulk per 8-step block on the
    otherwise-idle Pool engine, off the serial chain.
  - PE p-state warm-up: ~3us of junk matmuls issued while the first DMAs
    land, so real matmuls run at full clock from the start.
"""

import sys

for _p in ("/opt/trn_rl_repo",):
    if _p not in sys.path:
        sys.path.insert(0, _p)

import numpy as np
import ml_dtypes

import concourse.bass as bass
import concourse.mybir as mybir
import concourse.tile as tile
from concourse.bass_utils import run_bass_kernel_spmd

B, CIN, COUT, T = 64, 2048, 2048, 128
NCORES = 8
NB = 2                      # batch shards
NQ = 4                      # cout shards
BLOC = B // NB              # 32 samples per core
QLOC = COUT // NQ           # 512 output channels per core
MC = QLOC // 128            # 4 output-channel chunks per core
KC = CIN // 128             # 16 contraction chunks
TB = 8                      # timesteps per scan block
NBLK = T // TB              # 16 blocks
NFREE = BLOC * TB           # matmul free dim = 256

# spike DMA chunks along T (first small so PE starts early)
TCHUNKS = [8, 8, 16, 32, 32, 32]
assert sum(TCHUNKS) == T

_MAX_WAITS = 1


def _split_excess_waits(nc):
    """This container's walrus build accepts at most one sync-wait per
    instruction; spill extra waits onto same-engine NOPs placed before the
    offending instruction."""
    for f in nc.m.functions:
        for bb in f.blocks:
            new_list = []
            for ins in bb.instructions:
                si = ins.sync_info
                waits = list(si.on_wait) if si is not None and si.on_wait else []
                if len(waits) > _MAX_WAITS:
                    extra, keep = waits[:-_MAX_WAITS], waits[-_MAX_WAITS:]
                    for i in range(0, len(extra), _MAX_WAITS):
                        nop = mybir.InstNoOp(
                            name=f"{ins.name}-waitsplit-{i}", ins=[], outs=[]
                        )
                        nop.engine = ins.engine
                        nop.sync_info = mybir.SyncInfo(
                            on_wait=extra[i : i + _MAX_WAITS], on_update=[]
                        )
                        new_list.append(nop)
                    ins.sync_info = mybir.SyncInfo(
                        on_wait=keep,
                        on_update=list(si.on_update) if si.on_update else [],
                    )
                new_list.append(ins)
            bb.instructions[:] = new_list


def _build(thr: float):
    f32 = mybir.dt.float32
    bf16 = mybir.dt.bfloat16
    u8 = mybir.dt.uint8
    nc = bass.Bass()

    # W^T (2**scale_exp folded in): per m-chunk [cin_lo, k, cout_lo]
    wt_ds = [
        nc.dram_tensor(f"wt{m}", [128, KC, 128], bf16, kind="ExternalInput")
        for m in range(MC)
    ]
    # spike chunks, each contiguous [cin_lo, k, b, tc]
    spk_ds = [
        nc.dram_tensor(f"spk{j}", [128, KC, BLOC, tc], bf16, kind="ExternalInput")
        for j, tc in enumerate(TCHUNKS)
    ]
    # per-block spike outputs [cout_lo, t, m, b]
    out_ds = [
        nc.dram_tensor(f"out{j}", [128, TB, MC, BLOC], u8, kind="ExternalOutput")
        for j in range(NBLK)
    ]

    # block -> (chunk index, t offset within chunk)
    blk_map = []
    cj, coff = 0, 0
    for j in range(NBLK):
        if coff >= TCHUNKS[cj]:
            cj += 1
            coff = 0
        blk_map.append((cj, coff))
        coff += TB

    with tile.TileContext(nc) as tc:
        with (
            tc.tile_pool(name="const", bufs=1) as const,
            tc.tile_pool(name="cpool", bufs=3) as cpool,
            tc.tile_pool(name="upool", bufs=3) as upool,
            tc.tile_pool(name="opool", bufs=3) as opool,
            tc.tile_pool(name="psum", bufs=3, space="PSUM") as psum,
            tc.tile_pool(name="pwarm", bufs=1, space="PSUM") as pwarm,
        ):
            wt_sb = const.tile([128, MC, KC, 128], bf16)
            spk_sbs = [
                const.tile([128, KC, BLOC, tc], bf16, name=f"spk_sb{j}")
                for j, tc in enumerate(TCHUNKS)
            ]
            acc = const.tile([128, MC, BLOC], f32)
            junk = const.tile([128, 256], bf16)

            nc.vector.memset(acc[:], 0.0)
            nc.vector.memset(junk[:], 0.0)

            # DMA order: first spike chunk + weights first so PE starts ASAP
            nc.sync.dma_start(spk_sbs[0][:], spk_ds[0][:])
            for m in range(MC):
                nc.sync.dma_start(wt_sb[:, m], wt_ds[m][:])
            for j in range(1, len(TCHUNKS)):
                nc.sync.dma_start(spk_sbs[j][:], spk_ds[j][:])

            # PE p-state warm-up on junk data while the DMAs land (~3us)
            wps = pwarm.tile([128, 256], f32)
            for _ in range(22):
                nc.tensor.matmul(wps[:], lhsT=junk[:, :128], rhs=junk[:])

            for j in range(NBLK):
                cj, toff = blk_map[j]
                ps = psum.tile([128, MC, NFREE], f32, tag="ps", name="ps")
                for m in range(MC):
                    for k in range(KC):
                        nc.tensor.matmul(
                            ps[:, m, :],
                            lhsT=wt_sb[:, m, k, :],
                            rhs=spk_sbs[cj][:, k, :, toff : toff + TB],
                            start=(k == 0),
                            stop=(k == KC - 1),
                        )
                # PSUM [p, m, (b t)] -> SBUF c [p, t, m, b] so each scan step
                # reads a contiguous [128, (m b)] slice.  Drained in two
                # half-blocks so the scan can start while the second half
                # drains (shrinks the post-matmul tail).
                H = TB // 2
                c = cpool.tile([128, TB, MC, BLOC], f32, tag="cblk")
                ps_v = ps[:].rearrange("p m (b t) -> p m b t", b=BLOC)
                for h in range(2):
                    nc.scalar.copy(
                        c[:, h * H : (h + 1) * H].rearrange("p t m b -> p m b t"),
                        ps_v[:, :, :, h * H : (h + 1) * H],
                    )
                u = upool.tile([128, TB, MC, BLOC], f32, tag="ublk")
                ob = opool.tile([128, TB, MC, BLOC], u8, tag="oblk")
                for h in range(2):
                    for t in range(h * H, (h + 1) * H):
                        nc.vector.tensor_tensor(
                            u[:, t], acc[:], c[:, t], mybir.AluOpType.add
                        )
                        nc.vector.scalar_tensor_tensor(
                            acc[:], u[:, t], thr, u[:, t],
                            mybir.AluOpType.is_lt, mybir.AluOpType.mult,
                        )
                    # spikes = (u >= thr), per half-block off the serial chain
                    # (DVE: the Pool engine runs tensor_scalar ~10x slower
                    # than its nominal rate, measured 15.6us per block)
                    nc.vector.tensor_scalar(
                        ob[:, h * H : (h + 1) * H],
                        u[:, h * H : (h + 1) * H],
                        thr, None, mybir.AluOpType.is_ge,
                    )
                    nc.sync.dma_start(
                        out_ds[j][:, h * H : (h + 1) * H],
                        ob[:, h * H : (h + 1) * H],
                    )

    _split_excess_waits(nc)
    return nc


def _prep_inputs(spikes, weights, mask, scale_exp):
    wm = weights * mask  # integers <= 127, exact
    scale = np.exp2(scale_exp.astype(np.float64)).astype(np.float32)
    wm = (wm * scale[:, None]).astype(np.float32)  # fold power-of-2 scale in
    in_maps = []
    for core in range(NCORES):
        bh, cq = divmod(core, NQ)
        # weights for this cout shard: [qloc, cin] -> W^T -> [m, cin_lo, k, cout_lo]
        wq = wm[cq * QLOC : (cq + 1) * QLOC]  # [512, 2048]
        wt = (
            wq.T.reshape(KC, 128, MC, 128)
            .transpose(2, 1, 0, 3)
            .astype(ml_dtypes.bfloat16)
        )  # [m, cin_lo, k, cout_lo]
        m = {f"wt{mm}": np.ascontiguousarray(wt[mm]) for mm in range(MC)}
        # spikes for this batch shard: [b, cin, t] -> [cin_lo, k, b, t]
        s = spikes[bh * BLOC : (bh + 1) * BLOC]
        a = s.transpose(1, 0, 2).reshape(KC, 128, BLOC, T).transpose(1, 0, 2, 3)
        a = a.astype(ml_dtypes.bfloat16)
        t0 = 0
        for jj, tc in enumerate(TCHUNKS):
            m[f"spk{jj}"] = np.ascontiguousarray(a[:, :, :, t0 : t0 + tc])
            t0 += tc
        in_maps.append(m)
    return in_maps


_CACHE = {}


def _get_program(thr: float):
    if thr not in _CACHE:
        _CACHE[thr] = _build(thr)
    return _CACHE[thr]


def kernel(spikes, weights, mask, scale_exp, threshold_exp, **run_kwargs):
    thr = float(2.0 ** int(np.asarray(threshold_exp)))
    nc = _get_program(thr)
    in_maps = _prep_inputs(
        np.asarray(spikes, dtype=np.float32),
        np.asarray(weights, dtype=np.float32),
        np.asarray(mask, dtype=np.float32),
        np.asarray(scale_exp),
    )
    res = run_bass_kernel_spmd(
        nc, in_maps, core_ids=list(range(NCORES)), **run_kwargs
    )
    full = np.zeros((B, COUT, T), dtype=np.float32)
    for core in range(NCORES):
        bh, cq = divmod(core, NQ)
        blks = [
            np.asarray(res.results[core][f"out{j}"]) for j in range(NBLK)
        ]  # each [cout_lo, t, m, b]
        a = np.concatenate(blks, axis=1)  # [cout_lo, T, m, b]
        # -> [b, m, cout_lo, T] -> [b_loc, qloc, T]
        a = a.transpose(3, 2, 0, 1).reshape(BLOC, QLOC, T)
        full[bh * BLOC : (bh + 1) * BLOC, cq * QLOC : (cq + 1) * QLOC] = a
    if run_kwargs:
        return full, res
    return full


# revision 10
# speedup vs baseline: 2.3911x; 1.0200x over previous
"""Trainium2 Bass kernel for nn_HardwareOptimizedSpikeProcessor.

Reference semantics (per timestep t):
    acc += (s_t @ (W*mask).T) * 2**scale_exp     # [B, Cout]
    spk  = acc >= 2**threshold_exp
    acc  = acc * (1 - spk)
    out[:, :, t] = spk

Strategy (v2):
  - Shard batch/2 x cout/4: each of the 8 cores handles 32 samples x 512
    output channels.  PE work per core is unchanged (8.6 GFLOP bf16), but the
    matmul free dim is b*tb = 32*8 = 256 at a t-block of only 8 steps, which
    keeps LDWEIGHTS fully amortized while letting the sequential scan pipeline
    against the PE at fine (8-step) granularity -- the scan tail after the
    last matmul shrinks from ~60us (baseline) to ~6us.
  - The matmul contribution c[t] = s_t @ Wm.T is exact in bf16 (spikes are
    0/1; masked weights are ints in [-127,127]); PSUM accumulates fp32.
  - Scan step is 2 DVE instructions instead of 3:
        u_t  = acc + c_t                         (tensor_tensor add)
        acc  = (u_t < thr) * u_t                 (scalar_tensor_tensor)
    and spikes (u_t >= thr) are extracted in bulk per 8-step block on the
    otherwise-idle Pool engine, off the serial chain.
  - PE p-state warm-up: ~3us of junk matmuls issued while the first DMAs
    land, so real matmuls run at full clock from the start.
"""

import sys

for _p in ("/opt/trn_rl_repo",):
    if _p not in sys.path:
        sys.path.insert(0, _p)

import numpy as np
import ml_dtypes

import concourse.bass as bass
import concourse.mybir as mybir
import concourse.tile as tile
from concourse.bass_utils import run_bass_kernel_spmd

B, CIN, COUT, T = 64, 2048, 2048, 128
NCORES = 8
NB = 2                      # batch shards
NQ = 4                      # cout shards
BLOC = B // NB              # 32 samples per core
QLOC = COUT // NQ           # 512 output channels per core
MC = QLOC // 128            # 4 output-channel chunks per core
KC = CIN // 128             # 16 contraction chunks
TB = 8                      # timesteps per scan block
NBLK = T // TB              # 16 blocks
NFREE = BLOC * TB           # matmul free dim = 256

# spike DMA chunks along T (first small so PE starts early)
TCHUNKS = [8, 8, 16, 32, 32, 32]
assert sum(TCHUNKS) == T

_MAX_WAITS = 1


def _split_excess_waits(nc):
    """This container's walrus build accepts at most one sync-wait per
    instruction; spill extra waits onto same-engine NOPs placed before the
    offending instruction."""
    for f in nc.m.functions:
        for bb in f.blocks:
            new_list = []
            for ins in bb.instructions:
                si = ins.sync_info
                waits = list(si.on_wait) if si is not None and si.on_wait else []
                if len(waits) > _MAX_WAITS:
                    extra, keep = waits[:-_MAX_WAITS], waits[-_MAX_WAITS:]
                    for i in range(0, len(extra), _MAX_WAITS):
                        nop = mybir.InstNoOp(
                            name=f"{ins.name}-waitsplit-{i}", ins=[], outs=[]
                        )
                        nop.engine = ins.engine
                        nop.sync_info = mybir.SyncInfo(
                            on_wait=extra[i : i + _MAX_WAITS], on_update=[]
                        )
                        new_list.append(nop)
                    ins.sync_info = mybir.SyncInfo(
                        on_wait=keep,
                        on_update=list(si.on_update) if si.on_update else [],
                    )
                new_list.append(ins)
            bb.instructions[:] = new_list


def _build(thr: float):
    f32 = mybir.dt.float32
    bf16 = mybir.dt.bfloat16
    fp8 = mybir.dt.float8e4
    u8 = mybir.dt.uint8
    nc = bass.Bass()

    # W^T (2**scale_exp folded in): per m-chunk [cin_lo, k, cout_lo]
    wt_ds = [
        nc.dram_tensor(f"wt{m}", [128, KC, 128], bf16, kind="ExternalInput")
        for m in range(MC)
    ]
    # spike chunks, each contiguous [cin_lo, k, b, tc]
    spk_ds = [
        nc.dram_tensor(f"spk{j}", [128, KC, BLOC, tc], fp8, kind="ExternalInput")
        for j, tc in enumerate(TCHUNKS)
    ]
    # per-block spike outputs [cout_lo, t, m, b]
    out_ds = [
        nc.dram_tensor(f"out{j}", [128, TB, MC, BLOC], u8, kind="ExternalOutput")
        for j in range(NBLK)
    ]

    # block -> (chunk index, t offset within chunk)
    blk_map = []
    cj, coff = 0, 0
    for j in range(NBLK):
        if coff >= TCHUNKS[cj]:
            cj += 1
            coff = 0
        blk_map.append((cj, coff))
        coff += TB

    with tile.TileContext(nc) as tc:
        with (
            tc.tile_pool(name="const", bufs=1) as const,
            tc.tile_pool(name="cpool", bufs=3) as cpool,
            tc.tile_pool(name="upool", bufs=3) as upool,
            tc.tile_pool(name="opool", bufs=3) as opool,
            tc.tile_pool(name="psum", bufs=3, space="PSUM") as psum,
            tc.tile_pool(name="pwarm", bufs=1, space="PSUM") as pwarm,
        ):
            wt_sb = const.tile([128, MC, KC, 128], bf16)
            spk_sbs = [
                const.tile([128, KC, BLOC, tc], fp8, name=f"spk_sb{j}")
                for j, tc in enumerate(TCHUNKS)
            ]
            acc = const.tile([128, MC, BLOC], f32)
            junk = const.tile([128, 256], bf16)

            nc.vector.memset(acc[:], 0.0)
            nc.vector.memset(junk[:], 0.0)

            # DMA order: first spike chunk + weights first so PE starts ASAP
            nc.sync.dma_start(spk_sbs[0][:], spk_ds[0][:])
            for m in range(MC):
                nc.sync.dma_start(wt_sb[:, m], wt_ds[m][:])
            for j in range(1, len(TCHUNKS)):
                nc.sync.dma_start(spk_sbs[j][:], spk_ds[j][:])

            # PE p-state warm-up on junk data while the DMAs land (~3us)
            wps = pwarm.tile([128, 256], f32)
            for _ in range(14):
                nc.tensor.matmul(wps[:], lhsT=junk[:, :128], rhs=junk[:])

            for j in range(NBLK):
                cj, toff = blk_map[j]
                ps = psum.tile([128, MC, NFREE], f32, tag="ps", name="ps")
                for m in range(MC):
                    for k in range(KC):
                        nc.tensor.matmul(
                            ps[:, m, :],
                            lhsT=wt_sb[:, m, k, :],
                            rhs=spk_sbs[cj][:, k, :, toff : toff + TB],
                            start=(k == 0),
                            stop=(k == KC - 1),
                        )
                # PSUM [p, m, (b t)] -> SBUF c [p, t, m, b] so each scan step
                # reads a contiguous [128, (m b)] slice.  Drained in two
                # half-blocks so the scan can start while the second half
                # drains (shrinks the post-matmul tail).
                H = TB // 2
                last = j == NBLK - 1
                ps_v = ps[:].rearrange("p m (b t) -> p m b t", b=BLOC)
                if not last:
                    c = cpool.tile([128, TB, MC, BLOC], f32, tag="cblk")
                    for h in range(2):
                        nc.scalar.copy(
                            c[:, h * H : (h + 1) * H].rearrange(
                                "p t m b -> p m b t"
                            ),
                            ps_v[:, :, :, h * H : (h + 1) * H],
                        )
                u = upool.tile([128, TB, MC, BLOC], f32, tag="ublk")
                ob = opool.tile([128, TB, MC, BLOC], u8, tag="oblk")
                for h in range(2):
                    for t in range(h * H, (h + 1) * H):
                        # last block: read c straight from PSUM (skips the
                        # ACT drain latency right at the kernel tail)
                        c_t = ps_v[:, :, :, t] if last else c[:, t]
                        nc.vector.tensor_tensor(
                            u[:, t], acc[:], c_t, mybir.AluOpType.add
                        )
                        nc.vector.scalar_tensor_tensor(
                            acc[:], u[:, t], thr, u[:, t],
                            mybir.AluOpType.is_lt, mybir.AluOpType.mult,
                        )
                    # spikes = (u >= thr), per half-block off the serial chain
                    # (DVE: the Pool engine runs tensor_scalar ~10x slower
                    # than its nominal rate, measured 15.6us per block)
                    nc.vector.tensor_scalar(
                        ob[:, h * H : (h + 1) * H],
                        u[:, h * H : (h + 1) * H],
                        thr, None, mybir.AluOpType.is_ge,
                    )
                    nc.sync.dma_start(
                        out_ds[j][:, h * H : (h + 1) * H],
                        ob[:, h * H : (h + 1) * H],
                    )

    _split_excess_waits(nc)
    return nc


def _prep_inputs(spikes, weights, mask, scale_exp):
    wm = weights * mask  # integers <= 127, exact
    scale = np.exp2(scale_exp.astype(np.float64)).astype(np.float32)
    wm = (wm * scale[:, None]).astype(np.float32)  # fold power-of-2 scale in
    in_maps = []
    for core in range(NCORES):
        bh, cq = divmod(core, NQ)
        # weights for this cout shard: [qloc, cin] -> W^T -> [m, cin_lo, k, cout_lo]
        wq = wm[cq * QLOC : (cq + 1) * QLOC]  # [512, 2048]
        wt = (
            wq.T.reshape(KC, 128, MC, 128)
            .transpose(2, 1, 0, 3)
            .astype(ml_dtypes.bfloat16)
        )  # [m, cin_lo, k, cout_lo]
        m = {f"wt{mm}": np.ascontiguousarray(wt[mm]) for mm in range(MC)}
        # spikes for this batch shard: [b, cin, t] -> [cin_lo, k, b, t]
        s = spikes[bh * BLOC : (bh + 1) * BLOC]
        a = s.transpose(1, 0, 2).reshape(KC, 128, BLOC, T).transpose(1, 0, 2, 3)
        a = a.astype(ml_dtypes.float8_e4m3)
        t0 = 0
        for jj, tc in enumerate(TCHUNKS):
            m[f"spk{jj}"] = np.ascontiguousarray(a[:, :, :, t0 : t0 + tc])
            t0 += tc
        in_maps.append(m)
    return in_maps


_CACHE = {}


def _get_program(thr: float):
    if thr not in _CACHE:
        _CACHE[thr] = _build(thr)
    return _CACHE[thr]


def kernel(spikes, weights, mask, scale_exp, threshold_exp, **run_kwargs):
    thr = float(2.0 ** int(np.asarray(threshold_exp)))
    nc = _get_program(thr)
    in_maps = _prep_inputs(
        np.asarray(spikes, dtype=np.float32),
        np.asarray(weights, dtype=np.float32),
        np.asarray(mask, dtype=np.float32),
        np.asarray(scale_exp),
    )
    res = run_bass_kernel_spmd(
        nc, in_maps, core_ids=list(range(NCORES)), **run_kwargs
    )
    full = np.zeros((B, COUT, T), dtype=np.float32)
    for core in range(NCORES):
        bh, cq = divmod(core, NQ)
        blks = [
            np.asarray(res.results[core][f"out{j}"]) for j in range(NBLK)
        ]  # each [cout_lo, t, m, b]
        a = np.concatenate(blks, axis=1)  # [cout_lo, T, m, b]
        # -> [b, m, cout_lo, T] -> [b_loc, qloc, T]
        a = a.transpose(3, 2, 0, 1).reshape(BLOC, QLOC, T)
        full[bh * BLOC : (bh + 1) * BLOC, cq * QLOC : (cq + 1) * QLOC] = a
    if run_kwargs:
        return full, res
    return full
